# revision 1
# baseline (speedup 1.0000x reference)
"""Bass/TRN2 kernel for nn_CrossAttentionModel_20684562497797.

The reference computes q/k projections + RMSNorm + per-head all-pairs dot
products, then applies a softmax over a size-1 axis (`scores[..., None]`,
axis=-1) and averages over heads.  A softmax over a single element is
identically 1.0 (exp(x-x)/exp(x-x)), so the mean over heads is exactly 1.0
for every (i, j) pair regardless of the input values: the whole projection /
normalization / einsum pipeline is dead code and the reference output is
exactly np.ones((B1, B2), float32).

The kernel therefore shards the output rows across the 8 cores (data-parallel
over vectors_1 rows, per the sharding hint); each core materializes its
(B1/8, B2) slab of ones on-device (memset of one SBUF tile, then a single
broadcast-source HWDGE DMA to HBM) and the host concatenates the slabs.
"""

import sys

import numpy as np

if "/opt/trn_rl_repo" not in sys.path:
    sys.path.insert(0, "/opt/trn_rl_repo")

B1 = 2048
B2 = 2048
N_CORES = 8
ROWS_PER_CORE = B1 // N_CORES  # 256

_P = 128  # SBUF partitions
_TILE_F = 512  # free-dim elems memset per partition; DMA re-reads via step-0 AP

_cache: dict = {}


def _build_nc():
    import concourse.bass as bass
    import concourse.mybir as mybir

    nc = bass.Bass()
    out = nc.declare_dram_parameter(
        "out", [ROWS_PER_CORE, B2], mybir.dt.float32, isOutput=True
    )

    reps = (ROWS_PER_CORE * B2) // (_P * _TILE_F)

    with (
        nc.sbuf_tensor([_P, _TILE_F], mybir.dt.float32) as tile,
        nc.semaphore("vsem") as vsem,
        nc.semaphore("dsem") as dsem,
        nc.Block() as block,
    ):
        src = tile[:, None, :].to_broadcast((_P, reps, _TILE_F))

        @block.vector
        def _(vector):
            vector.memset(tile[:], 1.0).then_inc(vsem, 1)

        @block.sync
        def _(sync):
            sync.wait_ge(vsem, 1)
            sync.dma_start(out=out[:], in_=src).then_inc(dsem, 16)
            sync.wait_ge(dsem, 16)

    return nc


def kernel(**inputs: np.ndarray) -> np.ndarray:
    from concourse.bass_utils import run_bass_kernel_spmd

    assert inputs["vectors_1"].shape[0] == B1
    assert inputs["vectors_2"].shape[0] == B2

    if "nc" not in _cache:
        _cache["nc"] = _build_nc()

    res = run_bass_kernel_spmd(
        _cache["nc"], [{} for _ in range(N_CORES)], list(range(N_CORES))
    )
    return np.concatenate(
        [np.asarray(res.results[c]["out"]) for c in range(N_CORES)], axis=0
    )


# revision 4
# speedup vs baseline: 1.7792x; 1.7792x over previous
"""Bass/TRN2 kernel for nn_CrossAttentionModel_20684562497797.

The reference computes q/k projections + RMSNorm + per-head all-pairs dot
products, then applies a softmax over a size-1 axis (`scores[..., None]`,
axis=-1) and averages over heads.  A softmax over a single element is
identically 1.0 (exp(x-x)/exp(x-x)), so the mean over heads is exactly 1.0
for every (i, j) pair regardless of the input values: the whole projection /
normalization / einsum pipeline is dead code and the reference output is
exactly np.ones((B1, B2), float32).

The kernel therefore shards the output rows across the 8 cores (data-parallel
over vectors_1 rows, per the sharding hint); each core materializes its
(B1/8, B2) slab of ones on-device with a single broadcast-source HWDGE DMA
(a 2KB host-supplied block of ones, re-read via a step-0 access pattern,
written across the full 2MB slab) and the host concatenates the slabs.
The NEFF epilogue's queue drains guarantee DMA completion, so the transfer
overlaps the fixed end-of-kernel semaphore-reset sweep.
"""

import sys

import numpy as np

if "/opt/trn_rl_repo" not in sys.path:
    sys.path.insert(0, "/opt/trn_rl_repo")

B1 = 2048
B2 = 2048
N_CORES = 8
ROWS_PER_CORE = B1 // N_CORES  # 256

_BLK = 512  # f32 elems in the host-supplied ones block (2KB)

_cache: dict = {}


def _build_nc():
    import concourse.bass as bass
    import concourse.mybir as mybir

    nc = bass.Bass()
    ones_in = nc.declare_dram_parameter("ones", [_BLK], mybir.dt.float32, isOutput=False)
    out = nc.declare_dram_parameter(
        "out", [ROWS_PER_CORE, B2], mybir.dt.float32, isOutput=True
    )

    reps = (ROWS_PER_CORE * B2) // _BLK

    with (
        nc.semaphore("dsem") as dsem,
        nc.Block() as block,
    ):
        src = ones_in[None, :].to_broadcast((reps, _BLK))

        @block.sync
        def _(sync):
            sync.dma_start(out=out[:], in_=src).then_inc(dsem, 16)

    return nc


def _in_maps():
    ones_blk = np.ones([_BLK], dtype=np.float32)
    return [{"ones": ones_blk} for _ in range(N_CORES)]


def kernel(**inputs: np.ndarray) -> np.ndarray:
    from concourse.bass_utils import run_bass_kernel_spmd

    assert inputs["vectors_1"].shape[0] == B1
    assert inputs["vectors_2"].shape[0] == B2

    if "nc" not in _cache:
        _cache["nc"] = _build_nc()

    res = run_bass_kernel_spmd(
        _cache["nc"], _in_maps(), list(range(N_CORES))
    )
    return np.concatenate(
        [np.asarray(res.results[c]["out"]) for c in range(N_CORES)], axis=0
    )
